# revision 16
# baseline (speedup 1.0000x reference)
"""ContextualLoss on 8 Trainium2 NeuronCores (Bass/Tile).

Problem: nn_ContextualLoss — N=4, C=64, H=W=64, P=H*W=4096.

Math (per batch n):
  mu       = mean of T over (N,H,W)                              [C]
  Tc/Ic    = centered features;  h_p = 1/|Tc_p|, g_q = 1/|Ic_q|
  c[q,p]   = (Ic_q . Tc_p) * h_p * g_q   (cosine)                [P, P]
  mq       = max_p c ; a2 = 1/(1+2eps - mq); bias = 1-a2
  cs_w     = exp(a2*c + bias); cs = cs_w / sum_p cs_w
  k_p      = max_q cs ; CS_n = mean_p k_p ; score = mean_n(-log CS_n)

Sharding: 2 cores per batch; each core owns 2048 q rows (all 4096 p cols),
so row max/sum are core-local. Host rotates batches per core so each
core's batch T is flat rows 0-63 of the packed t_full layout — one SPMD
program for all cores.

The device computes, per 128-row block, eb = exp(a2*c + bias) [128, P]
fp16 plus the f32 row sums; eb streams to DRAM on the (otherwise idle)
DMA queues concurrently with compute. The host applies the cheap
normalize (eb * 1/ss) and the column max — the same reduction it already
performs across cores/blocks.

Prologue: T column norms are built from RAW T during the input DMA
(sum T^2 accumulated in PSUM per chunk) and corrected post-mu with
accumulating -2*mu.T@T matmuls; the tiny +|mu|^2 term is dropped
(4e-5 relative, far below fp16 noise). All Lns run before all Exps:
exactly two ACT table loads.

Main-loop engine assignment (per 128-row block):
  PE : 8 fp16 matmuls -> PSUM halves [128,2048]x2
  DVE: tensor_scalar per half (op0=mult by g_b, op1=max accum):
       dotb = psum*g_b -> SBUF fp16 AND rowmax in one 1x pass — frees
       PSUM immediately; + tiny chain (mq merge, a2, bias)
  ACT: exp per half [128,2048] fp16 SBUF with accum_out row sums
  DMA: eb halves stream out as soon as each exp half completes
"""

import numpy as np

import concourse.bacc as bacc_mod
import concourse.mybir as mybir
import concourse.tile as tile
from concourse.bass_utils import run_bass_kernel_spmd

N, C, H, W = 4, 64, 64, 64
P = H * W                  # 4096 template pixels
QH = P // 2                # 2048 query pixels per core
NBLK = QH // 128           # 16 q-blocks per core
NCORES = 8
EPS = 1e-5
F32 = mybir.dt.float32
F16 = mybir.dt.float16
AX = mybir.AxisListType
OP = mybir.AluOpType
AF = mybir.ActivationFunctionType

HW_ = P // 2               # psum half width


def build_nc():
    nc = bacc_mod.Bacc("TRN2", target_bir_lowering=False, debug=False)

    t_full = nc.dram_tensor("t_full", [128, 2 * P], F16, kind="ExternalInput")
    i_own = nc.dram_tensor("i_own", [C, QH], F16, kind="ExternalInput")
    eb_out = nc.dram_tensor("eb_out", [QH, P], F16, kind="ExternalOutput")
    ss_out = nc.dram_tensor("ss_out", [128, 2 * NBLK], F32,
                            kind="ExternalOutput")
    hbounce = nc.dram_tensor("hbounce", [1, P], F16, kind="Internal")

    with tile.TileContext(nc) as tc:
        with (
            tc.tile_pool(name="persist", bufs=1) as pp,
            tc.tile_pool(name="small", bufs=4) as sp,
        ):
            # ---------------- persistent tiles ----------------
            tn = pp.tile([C, P], F16)          # (T-mu) * h  (matmul rhs)
            ic = pp.tile([C, QH], F16)         # centered I (matmul lhsT)
            g = pp.tile([128, NBLK], F32)      # 1/|Ic_q| block-compact
            sscol = pp.tile([128, 2 * NBLK], F32)  # row sums (2 per block)
            ones64 = pp.tile([C, 1], F16)

            nc.vector.memset(ones64, 1.0)

            # ---------------- prologue ----------------
            with tc.tile_pool(name="pro", bufs=1) as pro:
                # warmup the Ln table while the DMA streams
                wrm = sp.tile([1, 8], F32, tag="wrm")
                nc.vector.memset(wrm, 1.0)
                nc.scalar.activation(out=wrm, in_=wrm, func=AF.Ln)

                # T stream + mean accumulation + raw squares/col sums.
                # partition p holds flat rows p and p+128 (row r = n*64+c).
                tf = pro.tile([128, 2 * P], F16, tag="tf")
                sq = pro.tile([C, 1024], F16, tag="sq")
                NCH = 8
                CW = 2 * P // NCH
                macc = sp.tile([128, NCH], F32, tag="macc")
                tsc = pro.tile([128, CW], F16, tag="tsc")
                lnh = sp.tile([1, P], F32, tag="lnh")
                sqi = pro.tile([C, QH], F16, tag="sqi")
                lng = sp.tile([128, NBLK], F32, tag="lng")
                iownb = pro.tile([C, QH], F16, tag="iownb")
                with tc.tile_pool(name="pps1", bufs=1, space="PSUM") as pps1:
                    psr = pps1.tile([1, P], F32, tag="psr")
                    for j in range(NCH):
                        nc.sync.dma_start(out=tf[:, j * CW:(j + 1) * CW],
                                          in_=t_full[:, j * CW:(j + 1) * CW])
                        nc.scalar.activation(out=tsc, in_=tf[:, j * CW:
                                                             (j + 1) * CW],
                                             func=AF.Copy,
                                             scale=-1.0 / (N * P),
                                             accum_out=macc[:, j:j + 1])
                        if j < 4:
                            # own batch: sum of raw T^2 into psr (term 1)
                            nc.vector.tensor_tensor(
                                out=sq, in0=tf[0:C, j * CW:(j + 1) * CW],
                                in1=tf[0:C, j * CW:(j + 1) * CW], op=OP.mult)
                            for h in range(2):
                                cs_ = slice(j * CW + h * 512,
                                            j * CW + h * 512 + 512)
                                nc.tensor.matmul(psr[:, cs_],
                                                 ones64, sq[:, h * 512:
                                                            (h + 1) * 512],
                                                 start=True, stop=False)
                    nc.sync.dma_start(out=iownb, in_=i_own[:, :])

                    ms = sp.tile([128, 1], F32, tag="ms")
                    nc.vector.reduce_sum(out=ms, in_=macc, axis=AX.X)
                    rot0 = sp.tile([C, 1], F32, tag="rot0")
                    nc.sync.dma_start(out=rot0, in_=ms[64:128, :])
                    negmu = sp.tile([C, 1], F32, tag="negmu")
                    nc.vector.tensor_tensor(out=negmu, in0=ms[0:C, 0:1],
                                            in1=rot0, op=OP.add)
                    n2mu = sp.tile([C, 1], F16, tag="n2mu")
                    nc.vector.tensor_scalar_mul(n2mu, negmu, 2.0)

                    # term 2: psr += sum_c (-2 mu_c) T[c,p] (drop +|mu|^2)
                    for j in range(NCH):
                        cs_ = slice(j * 512, (j + 1) * 512)
                        nc.tensor.matmul(psr[:, cs_], n2mu, tf[0:C, cs_],
                                         start=False, stop=True)
                    nc.scalar.activation(out=lnh, in_=psr, func=AF.Ln)

                    # I side: center (correct sign) and squares meanwhile
                    nc.vector.tensor_scalar(out=ic, in0=iownb,
                                            scalar1=ms[0:C, 0:1],
                                            scalar2=rot0,
                                            op0=OP.add, op1=OP.add)
                    nc.vector.tensor_tensor(out=sqi, in0=ic, in1=ic,
                                            op=OP.mult)

                with tc.tile_pool(name="pps2", bufs=1, space="PSUM") as pps2:
                    g2 = pps2.tile([128, NBLK], F32, tag="g2")
                    for b in range(NBLK):
                        nc.tensor.matmul(g2[:, b:b + 1],
                                         sqi[:, b * 128:(b + 1) * 128],
                                         ones64, start=True, stop=True)
                    nc.scalar.activation(out=lng, in_=g2, func=AF.Ln)

                # T center for tn (off the h critical path)
                tcent = pro.tile([C, P], F16, tag="tcent")
                nc.vector.tensor_scalar(out=tcent, in0=tf[0:C, 0:P],
                                        scalar1=ms[0:C, 0:1], scalar2=rot0,
                                        op0=OP.add, op1=OP.add)

                # single table switch: all Exps (g first — it gates block 0)
                nc.scalar.activation(out=g, in_=lng, func=AF.Exp, scale=-0.5)
                ht = pro.tile([1, P], F16, tag="ht")
                hbc = pro.tile([C, P], F16, tag="hbc")
                for q in range(4):
                    qs = slice(q * 1024, (q + 1) * 1024)
                    nc.scalar.activation(out=ht[:, qs], in_=lnh[:, qs],
                                         func=AF.Exp, scale=-0.5)
                    # broadcast to C partitions: DRAM bounce + 0-stride read
                    nc.sync.dma_start(out=hbounce[0:1, qs], in_=ht[:, qs])
                    nc.sync.dma_start(
                        out=hbc[:, qs],
                        in_=hbounce[0:1, qs].broadcast_to([C, 1024]))
                    nc.vector.tensor_tensor(out=tn[:, qs], in0=tcent[:, qs],
                                            in1=hbc[:, qs], op=OP.mult)

            # ---------------- main loop ----------------
            with (
                tc.tile_pool(name="dbuf", bufs=2) as dp,
                tc.tile_pool(name="ebuf", bufs=3) as ep,
                tc.tile_pool(name="mps", bufs=1, space="PSUM") as mps,
            ):
                for b in range(NBLK):
                    lhs = ic[:, b * 128:(b + 1) * 128]
                    dotb = dp.tile([128, P], F16, tag="dotb")
                    rm2 = sp.tile([128, 2], F32, tag="rm2")
                    for h in range(2):
                        ps = mps.tile([128, HW_], F32, tag=f"ps{h}")
                        for cch in range(HW_ // 512):
                            off = h * HW_ + cch * 512
                            nc.tensor.matmul(
                                ps[:, cch * 512:(cch + 1) * 512], lhs,
                                tn[:, off:off + 512], start=True, stop=True)
                        # fused: dotb = psum * g_b (fp16) + rowmax accum
                        nc.vector.tensor_scalar(
                            out=dotb[:, h * HW_:(h + 1) * HW_], in0=ps,
                            scalar1=g[:, b:b + 1], scalar2=None,
                            op0=OP.mult, op1=OP.max,
                            accum_out=rm2[:, h:h + 1])
                    mq = sp.tile([128, 1], F32, tag="mq")
                    nc.vector.reduce_max(out=mq, in_=rm2, axis=AX.X)
                    dd = sp.tile([128, 1], F32, tag="dd")
                    nc.vector.tensor_scalar(out=dd, in0=mq, scalar1=-1.0,
                                            scalar2=1.0 + 2.0 * EPS,
                                            op0=OP.mult, op1=OP.add)
                    a2 = sp.tile([128, 1], F32, tag="a2")
                    nc.vector.reciprocal(a2, dd)
                    bias = sp.tile([128, 1], F32, tag="bias")
                    nc.vector.tensor_scalar(out=bias, in0=a2, scalar1=-1.0,
                                            scalar2=1.0, op0=OP.mult,
                                            op1=OP.add)
                    eb = ep.tile([128, P], F16, tag="eb")
                    for h in range(2):
                        hs = slice(h * HW_, (h + 1) * HW_)
                        nc.scalar.activation(
                            out=eb[:, hs], in_=dotb[:, hs], func=AF.Exp,
                            bias=bias, scale=a2,
                            accum_out=sscol[:, 2 * b + h:2 * b + h + 1])
                        nc.sync.dma_start(
                            out=eb_out[b * 128:(b + 1) * 128, hs],
                            in_=eb[:, hs])

            nc.sync.dma_start(out=ss_out[:, :], in_=sscol)

    nc.compile()
    return nc


_NC_CACHE = {}


def _get_nc():
    if "nc" not in _NC_CACHE:
        _NC_CACHE["nc"] = build_nc()
    return _NC_CACHE["nc"]


def make_in_maps(I_features, T_features):
    I4 = np.asarray(I_features, dtype=np.float32).reshape(N, C, P)
    T4 = np.asarray(T_features, dtype=np.float32).reshape(N, C, P)
    I4 = I4.astype(np.float16)
    T4 = T4.astype(np.float16)
    in_maps = []
    for core in range(NCORES):
        n, half = core // 2, core % 2
        # rotate batches so this core's batch is flat rows 0-63; mu is
        # order-invariant. partition p holds flat rows p and p+128.
        perm = [(n + j) % N for j in range(N)]
        tf = np.ascontiguousarray(
            T4[perm].reshape(2, 128, P).transpose(1, 0, 2).reshape(128, 2 * P))
        in_maps.append({
            "t_full": tf,
            "i_own": np.ascontiguousarray(I4[n][:, half * QH:(half + 1) * QH]),
        })
    return in_maps


def core_k(eb, ss):
    """One core's partial column max [128, P] from eb [QH,P], ss [128,2*NBLK]."""
    eb3 = np.asarray(eb, dtype=np.float32).reshape(NBLK, 128, P)
    ss2 = np.asarray(ss, dtype=np.float32).reshape(128, NBLK, 2).sum(axis=2)
    rr = 1.0 / ss2                                     # [128, NBLK]
    return (eb3 * rr.T[:, :, None]).max(axis=0)        # [128, P]


def finish_host(kparts):
    """kparts: [8, 128, P] per-core partial column maxima -> scalar score."""
    ks = np.stack([np.asarray(kp, dtype=np.float64) for kp in kparts])
    kp = ks.reshape(N, 2 * 128, P).max(axis=1)      # [N, P]
    cs = kp.mean(axis=1)                            # [N]
    return np.float32(np.mean(-np.log(cs)))


def kernel(I_features, T_features, _trace=False):
    nc = _get_nc()
    in_maps = make_in_maps(I_features, T_features)
    res = run_bass_kernel_spmd(nc, in_maps, core_ids=list(range(NCORES)),
                               trace=_trace)
    score = finish_host([core_k(r["eb_out"], r["ss_out"])
                         for r in res.results])
    if _trace:
        return np.array(score, dtype=np.float32), res
    return np.array(score, dtype=np.float32)


# revision 17
# speedup vs baseline: 1.2146x; 1.2146x over previous
"""ContextualLoss on 8 Trainium2 NeuronCores (Bass/Tile).

Problem: nn_ContextualLoss — N=4, C=64, H=W=64, P=H*W=4096.

Math (per batch n):
  mu       = mean of T over (N,H,W)                              [C]
  Tc/Ic    = centered features;  h_p = 1/|Tc_p|, g_q = 1/|Ic_q|
  c[q,p]   = (Ic_q . Tc_p) * h_p * g_q   (cosine)                [P, P]
  mq       = max_p c ; a2 = 1/(1+2eps - mq); bias = 1-a2
  cs_w     = exp(a2*c + bias); cs = cs_w / sum_p cs_w
  k_p      = max_q cs ; CS_n = mean_p k_p ; score = mean_n(-log CS_n)

Sharding: 2 cores per batch; each core owns 2048 q rows (all 4096 p cols),
so row max/sum are core-local. Host rotates batches per core so each
core's batch T is flat rows 0-63 of the packed t_full layout — one SPMD
program for all cores.

The device computes, per 128-row block, eb = exp(a2*c + bias) [128, P]
fp16 plus the f32 row sums; eb streams to DRAM on the (otherwise idle)
DMA queues concurrently with compute. The host applies the cheap
normalize (eb * 1/ss) and the column max — the same reduction it already
performs across cores/blocks.

Prologue: T column norms are built from RAW T during the input DMA
(sum T^2 accumulated in PSUM per chunk) and corrected post-mu with
accumulating -2*mu.T@T matmuls (split into the ms and rot halves so the
first half starts before the cross-partition rotate DMA lands); the tiny
+|mu|^2 term is dropped (4e-5 relative, far below fp16 noise). All Lns
run before all Exps: exactly two ACT table loads. The h chain is
pipelined per quarter: Ln -> Exp -> DRAM bounce -> broadcast -> tn.

Main-loop engine assignment (per 128-row block):
  PE : 8 fp16 matmuls -> PSUM quarters [128,1024]x4
  DVE: tensor_scalar per quarter (op0=mult by g_b, op1=max accum):
       dotb = psum*g_b -> SBUF fp16 AND rowmax in one 1x pass — frees
       PSUM immediately; + tiny chain (mq merge, a2, bias)
  ACT: exp per half [128,2048] fp16 SBUF with accum_out row sums
  DMA: eb halves stream out as soon as each exp half completes
"""

import numpy as np

import concourse.bacc as bacc_mod
import concourse.mybir as mybir
import concourse.tile as tile
from concourse.bass_utils import run_bass_kernel_spmd

N, C, H, W = 4, 64, 64, 64
P = H * W                  # 4096 template pixels
QH = P // 2                # 2048 query pixels per core
NBLK = QH // 128           # 16 q-blocks per core
NCORES = 8
EPS = 1e-5
F32 = mybir.dt.float32
F16 = mybir.dt.float16
AX = mybir.AxisListType
OP = mybir.AluOpType
AF = mybir.ActivationFunctionType

HW_ = P // 2               # exp half width
QW = P // 4                # psum quarter width


def build_nc():
    nc = bacc_mod.Bacc("TRN2", target_bir_lowering=False, debug=False)

    t_full = nc.dram_tensor("t_full", [128, 2 * P], F16, kind="ExternalInput")
    i_own = nc.dram_tensor("i_own", [C, QH], F16, kind="ExternalInput")
    eb_out = nc.dram_tensor("eb_out", [QH, P], F16, kind="ExternalOutput")
    ss_out = nc.dram_tensor("ss_out", [128, 2 * NBLK], F32,
                            kind="ExternalOutput")
    hbounce = nc.dram_tensor("hbounce", [1, P], F16, kind="Internal")

    with tile.TileContext(nc) as tc:
        with (
            tc.tile_pool(name="persist", bufs=1) as pp,
            tc.tile_pool(name="small", bufs=4) as sp,
        ):
            # ---------------- persistent tiles ----------------
            tn = pp.tile([C, P], F16)          # (T-mu) * h  (matmul rhs)
            ic = pp.tile([C, QH], F16)         # centered I (matmul lhsT)
            g = pp.tile([128, NBLK], F32)      # 1/|Ic_q| block-compact
            sscol = pp.tile([128, 2 * NBLK], F32)  # row sums (2 per block)
            ones64 = pp.tile([C, 1], F16)

            nc.vector.memset(ones64, 1.0)

            # ---------------- prologue ----------------
            with tc.tile_pool(name="pro", bufs=1) as pro:
                # warmup the Ln table while the DMA streams
                wrm = sp.tile([1, 8], F32, tag="wrm")
                nc.vector.memset(wrm, 1.0)
                nc.scalar.activation(out=wrm, in_=wrm, func=AF.Ln)

                # T stream + mean accumulation + raw squares/col sums.
                # partition p holds flat rows p and p+128 (row r = n*64+c).
                tf = pro.tile([128, 2 * P], F16, tag="tf")
                sq = pro.tile([C, 1024], F16, tag="sq")
                NCH = 8
                CW = 2 * P // NCH
                macc = sp.tile([128, NCH], F32, tag="macc")
                tsc = pro.tile([128, CW], F16, tag="tsc")
                lnh = sp.tile([1, P], F32, tag="lnh")
                sqi = pro.tile([C, QH], F16, tag="sqi")
                lng = sp.tile([128, NBLK], F32, tag="lng")
                iownb = pro.tile([C, QH], F16, tag="iownb")
                with tc.tile_pool(name="pps1", bufs=1, space="PSUM") as pps1:
                    psr = pps1.tile([1, P], F32, tag="psr")
                    for j in range(NCH):
                        nc.sync.dma_start(out=tf[:, j * CW:(j + 1) * CW],
                                          in_=t_full[:, j * CW:(j + 1) * CW])
                        nc.scalar.activation(out=tsc, in_=tf[:, j * CW:
                                                             (j + 1) * CW],
                                             func=AF.Copy,
                                             scale=-1.0 / (N * P),
                                             accum_out=macc[:, j:j + 1])
                        if j < 4:
                            # own batch: sum of raw T^2 into psr (term 1)
                            nc.vector.tensor_tensor(
                                out=sq, in0=tf[0:C, j * CW:(j + 1) * CW],
                                in1=tf[0:C, j * CW:(j + 1) * CW], op=OP.mult)
                            for h in range(2):
                                cs_ = slice(j * CW + h * 512,
                                            j * CW + h * 512 + 512)
                                nc.tensor.matmul(psr[:, cs_],
                                                 ones64, sq[:, h * 512:
                                                            (h + 1) * 512],
                                                 start=True, stop=False)
                    nc.sync.dma_start(out=iownb, in_=i_own[:, :])

                    ms = sp.tile([128, 1], F32, tag="ms")
                    nc.vector.reduce_sum(out=ms, in_=macc, axis=AX.X)
                    rot0 = sp.tile([C, 1], F32, tag="rot0")
                    nc.sync.dma_start(out=rot0, in_=ms[64:128, :])
                    # term 2a: psr += sum_c 2*ms_c T[c,p] (ms = -mu half 1;
                    # starts before the rot0 DMA lands)
                    m2a = sp.tile([C, 1], F16, tag="m2a")
                    nc.vector.tensor_scalar_mul(m2a, ms[0:C, 0:1], 2.0)
                    for j in range(NCH):
                        cs_ = slice(j * 512, (j + 1) * 512)
                        nc.tensor.matmul(psr[:, cs_], m2a, tf[0:C, cs_],
                                         start=False, stop=False)
                    negmu = sp.tile([C, 1], F32, tag="negmu")
                    nc.vector.tensor_tensor(out=negmu, in0=ms[0:C, 0:1],
                                            in1=rot0, op=OP.add)
                    # term 2b: the rot half (drop +|mu|^2)
                    m2b = sp.tile([C, 1], F16, tag="m2b")
                    nc.vector.tensor_scalar_mul(m2b, rot0, 2.0)
                    for j in range(NCH):
                        cs_ = slice(j * 512, (j + 1) * 512)
                        nc.tensor.matmul(psr[:, cs_], m2b, tf[0:C, cs_],
                                         start=False, stop=True)
                    # all Lns (quartered so Exp/bcast/tn pipeline early)
                    for q in range(4):
                        qs = slice(q * 1024, (q + 1) * 1024)
                        nc.scalar.activation(out=lnh[:, qs], in_=psr[:, qs],
                                             func=AF.Ln)

                    # I side: center (correct sign) and squares meanwhile
                    nc.vector.tensor_scalar(out=ic, in0=iownb,
                                            scalar1=ms[0:C, 0:1],
                                            scalar2=rot0,
                                            op0=OP.add, op1=OP.add)
                    nc.vector.tensor_tensor(out=sqi, in0=ic, in1=ic,
                                            op=OP.mult)

                with tc.tile_pool(name="pps2", bufs=1, space="PSUM") as pps2:
                    g2 = pps2.tile([128, NBLK], F32, tag="g2")
                    for b in range(NBLK):
                        nc.tensor.matmul(g2[:, b:b + 1],
                                         sqi[:, b * 128:(b + 1) * 128],
                                         ones64, start=True, stop=True)
                    nc.scalar.activation(out=lng, in_=g2, func=AF.Ln)

                # T center for tn (off the h critical path)
                tcent = pro.tile([C, P], F16, tag="tcent")
                nc.vector.tensor_scalar(out=tcent, in0=tf[0:C, 0:P],
                                        scalar1=ms[0:C, 0:1], scalar2=rot0,
                                        op0=OP.add, op1=OP.add)

                # single table switch: all Exps (g first — it gates block 0)
                nc.scalar.activation(out=g, in_=lng, func=AF.Exp, scale=-0.5)
                ht = pro.tile([1, P], F16, tag="ht")
                hbc = pro.tile([C, P], F16, tag="hbc")
                for q in range(4):
                    qs = slice(q * 1024, (q + 1) * 1024)
                    nc.scalar.activation(out=ht[:, qs], in_=lnh[:, qs],
                                         func=AF.Exp, scale=-0.5)
                    # broadcast to C partitions: DRAM bounce + 0-stride read
                    nc.sync.dma_start(out=hbounce[0:1, qs], in_=ht[:, qs])
                    nc.sync.dma_start(
                        out=hbc[:, qs],
                        in_=hbounce[0:1, qs].broadcast_to([C, 1024]))
                    nc.vector.tensor_tensor(out=tn[:, qs], in0=tcent[:, qs],
                                            in1=hbc[:, qs], op=OP.mult)

            # ---------------- main loop ----------------
            with (
                tc.tile_pool(name="dbuf", bufs=3) as dp,
                tc.tile_pool(name="ebuf", bufs=3) as ep,
                tc.tile_pool(name="mps", bufs=1, space="PSUM") as mps,
            ):
                for b in range(NBLK):
                    lhs = ic[:, b * 128:(b + 1) * 128]
                    dotb = dp.tile([128, P], F16, tag="dotb")
                    rm4 = sp.tile([128, 4], F32, tag="rm4")
                    for q in range(4):
                        ps = mps.tile([128, QW], F32, tag=f"ps{q}")
                        for cch in range(QW // 512):
                            off = q * QW + cch * 512
                            nc.tensor.matmul(
                                ps[:, cch * 512:(cch + 1) * 512], lhs,
                                tn[:, off:off + 512], start=True, stop=True)
                        # fused: dotb = psum * g_b (fp16) + rowmax accum
                        nc.vector.tensor_scalar(
                            out=dotb[:, q * QW:(q + 1) * QW], in0=ps,
                            scalar1=g[:, b:b + 1], scalar2=None,
                            op0=OP.mult, op1=OP.max,
                            accum_out=rm4[:, q:q + 1])
                    mq = sp.tile([128, 1], F32, tag="mq")
                    nc.vector.reduce_max(out=mq, in_=rm4, axis=AX.X)
                    dd = sp.tile([128, 1], F32, tag="dd")
                    nc.vector.tensor_scalar(out=dd, in0=mq, scalar1=-1.0,
                                            scalar2=1.0 + 2.0 * EPS,
                                            op0=OP.mult, op1=OP.add)
                    a2 = sp.tile([128, 1], F32, tag="a2")
                    nc.vector.reciprocal(a2, dd)
                    bias = sp.tile([128, 1], F32, tag="bias")
                    nc.vector.tensor_scalar(out=bias, in0=a2, scalar1=-1.0,
                                            scalar2=1.0, op0=OP.mult,
                                            op1=OP.add)
                    eb = ep.tile([128, P], F16, tag="eb")
                    for h in range(2):
                        hs = slice(h * HW_, (h + 1) * HW_)
                        nc.scalar.activation(
                            out=eb[:, hs], in_=dotb[:, hs], func=AF.Exp,
                            bias=bias, scale=a2,
                            accum_out=sscol[:, 2 * b + h:2 * b + h + 1])
                        nc.sync.dma_start(
                            out=eb_out[b * 128:(b + 1) * 128, hs],
                            in_=eb[:, hs])

            nc.sync.dma_start(out=ss_out[:, :], in_=sscol)

    nc.compile()
    return nc


_NC_CACHE = {}


def _get_nc():
    if "nc" not in _NC_CACHE:
        _NC_CACHE["nc"] = build_nc()
    return _NC_CACHE["nc"]


def make_in_maps(I_features, T_features):
    I4 = np.asarray(I_features, dtype=np.float32).reshape(N, C, P)
    T4 = np.asarray(T_features, dtype=np.float32).reshape(N, C, P)
    I4 = I4.astype(np.float16)
    T4 = T4.astype(np.float16)
    in_maps = []
    for core in range(NCORES):
        n, half = core // 2, core % 2
        # rotate batches so this core's batch is flat rows 0-63; mu is
        # order-invariant. partition p holds flat rows p and p+128.
        perm = [(n + j) % N for j in range(N)]
        tf = np.ascontiguousarray(
            T4[perm].reshape(2, 128, P).transpose(1, 0, 2).reshape(128, 2 * P))
        in_maps.append({
            "t_full": tf,
            "i_own": np.ascontiguousarray(I4[n][:, half * QH:(half + 1) * QH]),
        })
    return in_maps


def core_k(eb, ss):
    """One core's partial column max [128, P] from eb [QH,P], ss [128,2*NBLK]."""
    eb3 = np.asarray(eb, dtype=np.float32).reshape(NBLK, 128, P)
    ss2 = np.asarray(ss, dtype=np.float32).reshape(128, NBLK, 2).sum(axis=2)
    rr = 1.0 / ss2                                     # [128, NBLK]
    return (eb3 * rr.T[:, :, None]).max(axis=0)        # [128, P]


def finish_host(kparts):
    """kparts: [8, 128, P] per-core partial column maxima -> scalar score."""
    ks = np.stack([np.asarray(kp, dtype=np.float64) for kp in kparts])
    kp = ks.reshape(N, 2 * 128, P).max(axis=1)      # [N, P]
    cs = kp.mean(axis=1)                            # [N]
    return np.float32(np.mean(-np.log(cs)))


def kernel(I_features, T_features, _trace=False):
    nc = _get_nc()
    in_maps = make_in_maps(I_features, T_features)
    res = run_bass_kernel_spmd(nc, in_maps, core_ids=list(range(NCORES)),
                               trace=_trace)
    score = finish_host([core_k(r["eb_out"], r["ss_out"])
                         for r in res.results])
    if _trace:
        return np.array(score, dtype=np.float32), res
    return np.array(score, dtype=np.float32)


# revision 19
# speedup vs baseline: 1.2347x; 1.0166x over previous
"""ContextualLoss on 8 Trainium2 NeuronCores (Bass/Tile).

Problem: nn_ContextualLoss — N=4, C=64, H=W=64, P=H*W=4096.

Math (per batch n):
  mu       = mean of T over (N,H,W)                              [C]
  Tc/Ic    = centered features;  h_p = 1/|Tc_p|, g_q = 1/|Ic_q|
  c[q,p]   = (Ic_q . Tc_p) * h_p * g_q   (cosine)                [P, P]
  mq       = max_p c ; a2 = 1/(1+2eps - mq); bias = 1-a2
  cs_w     = exp(a2*c + bias); cs = cs_w / sum_p cs_w
  k_p      = max_q cs ; CS_n = mean_p k_p ; score = mean_n(-log CS_n)

Sharding: 2 cores per batch; each core owns 2048 q rows (all 4096 p cols),
so row max/sum are core-local. Host rotates batches per core so each
core's batch T is flat rows 0-63 of the packed t_full layout — one SPMD
program for all cores.

The device computes, per 128-row block, eb = exp(a2*c + bias) [128, P]
fp16 plus the f32 row sums; eb streams to DRAM on the (otherwise idle)
DMA queues concurrently with compute. The host applies the cheap
normalize (eb * 1/ss) and the column max — the same reduction it already
performs across cores/blocks.

Prologue: T column norms are built from RAW T during the input DMA
(sum T^2 accumulated in PSUM per chunk) and corrected post-mu with
accumulating -2*mu.T@T matmuls (split into the ms and rot halves so the
first half starts before the cross-partition rotate DMA lands); the tiny
+|mu|^2 term is dropped (4e-5 relative, far below fp16 noise). All Lns
run before all Exps: exactly two ACT table loads. The h chain is
pipelined per quarter: Ln -> Exp -> DRAM bounce -> broadcast -> tn.

Main-loop engine assignment (per 128-row block):
  PE : 8 fp16 matmuls -> PSUM quarters [128,1024]x4
  DVE: tensor_scalar per quarter (op0=mult by g_b, op1=max accum):
       dotb = psum*g_b -> SBUF fp16 AND rowmax in one 1x pass — frees
       PSUM immediately; + tiny chain (mq merge, a2, bias)
  ACT: exp per half [128,2048] fp16 SBUF with accum_out row sums
  DMA: eb halves stream out as soon as each exp half completes
"""

import numpy as np

import concourse.bacc as bacc_mod
import concourse.mybir as mybir
import concourse.tile as tile
from concourse.bass_utils import run_bass_kernel_spmd

N, C, H, W = 4, 64, 64, 64
P = H * W                  # 4096 template pixels
QH = P // 2                # 2048 query pixels per core
NBLK = QH // 128           # 16 q-blocks per core
NCORES = 8
EPS = 1e-5
F32 = mybir.dt.float32
F16 = mybir.dt.float16
AX = mybir.AxisListType
OP = mybir.AluOpType
AF = mybir.ActivationFunctionType

HW_ = P // 2               # exp half width
QW = P // 4                # psum quarter width


def build_nc():
    nc = bacc_mod.Bacc("TRN2", target_bir_lowering=False, debug=False)

    t_full = nc.dram_tensor("t_full", [128, 2 * P], F16, kind="ExternalInput")
    i_own = nc.dram_tensor("i_own", [C, QH], F16, kind="ExternalInput")
    eb_out = nc.dram_tensor("eb_out", [QH, P], F16, kind="ExternalOutput")
    ss_out = nc.dram_tensor("ss_out", [128, 2 * NBLK], F32,
                            kind="ExternalOutput")
    hbounce = nc.dram_tensor("hbounce", [1, P], F16, kind="Internal")

    with tile.TileContext(nc) as tc:
        with (
            tc.tile_pool(name="persist", bufs=1) as pp,
            tc.tile_pool(name="small", bufs=4) as sp,
        ):
            # ---------------- persistent tiles ----------------
            tn = pp.tile([C, P], F16)          # (T-mu) * h  (matmul rhs)
            ic = pp.tile([C, QH], F16)         # centered I (matmul lhsT)
            g = pp.tile([128, NBLK], F32)      # 1/|Ic_q| block-compact
            sscol = pp.tile([128, 2 * NBLK], F32)  # row sums (2 per block)
            ones64 = pp.tile([C, 1], F16)

            nc.vector.memset(ones64, 1.0)

            # ---------------- prologue ----------------
            with tc.tile_pool(name="pro", bufs=1) as pro:
                # warmup the Ln table while the DMA streams
                wrm = sp.tile([1, 8], F32, tag="wrm")
                nc.vector.memset(wrm, 1.0)
                nc.scalar.activation(out=wrm, in_=wrm, func=AF.Ln)

                # T stream + mean accumulation + raw squares/col sums.
                # partition p holds flat rows p and p+128 (row r = n*64+c).
                tf = pro.tile([128, 2 * P], F16, tag="tf")
                sq = pro.tile([C, 1024], F16, tag="sq")
                NCH = 8
                CW = 2 * P // NCH
                macc = sp.tile([128, NCH], F32, tag="macc")
                tsc = pro.tile([128, CW], F16, tag="tsc")
                lnh = sp.tile([1, P], F32, tag="lnh")
                sqi = pro.tile([C, QH], F16, tag="sqi")
                lng = sp.tile([128, NBLK], F32, tag="lng")
                iownb = pro.tile([C, QH], F16, tag="iownb")
                with (
                    tc.tile_pool(name="pps1", bufs=1, space="PSUM") as pps1,
                    tc.tile_pool(name="pps2", bufs=1, space="PSUM") as pps2,
                ):
                    # psr quarters: 3 tags (6 banks) so g2's bank coexists;
                    # quarter 3 reuses tag 0 post-mu.
                    psrq = [None] * 4
                    for j in range(NCH):
                        nc.sync.dma_start(out=tf[:, j * CW:(j + 1) * CW],
                                          in_=t_full[:, j * CW:(j + 1) * CW])
                        nc.scalar.activation(out=tsc, in_=tf[:, j * CW:
                                                             (j + 1) * CW],
                                             func=AF.Copy,
                                             scale=-1.0 / (N * P),
                                             accum_out=macc[:, j:j + 1])
                        if j < 3:
                            # own batch: sum of raw T^2 (term 1)
                            pq_t = pps1.tile([1, 1024], F32,
                                             tag=f"pq{j}")
                            psrq[j] = pq_t
                            nc.vector.tensor_tensor(
                                out=sq, in0=tf[0:C, j * CW:(j + 1) * CW],
                                in1=tf[0:C, j * CW:(j + 1) * CW], op=OP.mult)
                            for h in range(2):
                                nc.tensor.matmul(
                                    psrq[j][:, h * 512:(h + 1) * 512],
                                    ones64, sq[:, h * 512:(h + 1) * 512],
                                    start=True, stop=False)
                    nc.sync.dma_start(out=iownb, in_=i_own[:, :])

                    ms = sp.tile([128, 1], F32, tag="ms")
                    nc.vector.reduce_sum(out=ms, in_=macc, axis=AX.X)
                    rot0 = sp.tile([C, 1], F32, tag="rot0")
                    nc.sync.dma_start(out=rot0, in_=ms[64:128, :])
                    negmu = sp.tile([C, 1], F32, tag="negmu")
                    nc.vector.tensor_tensor(out=negmu, in0=ms[0:C, 0:1],
                                            in1=rot0, op=OP.add)
                    n2mu = sp.tile([C, 1], F16, tag="n2mu")
                    nc.vector.tensor_scalar_mul(n2mu, negmu, 2.0)

                    # I side: center (correct sign) and squares; T center
                    nc.vector.tensor_scalar(out=ic, in0=iownb,
                                            scalar1=ms[0:C, 0:1],
                                            scalar2=rot0,
                                            op0=OP.add, op1=OP.add)
                    nc.vector.tensor_tensor(out=sqi, in0=ic, in1=ic,
                                            op=OP.mult)
                    tcent = pro.tile([C, P], F16, tag="tcent")
                    nc.vector.tensor_scalar(out=tcent, in0=tf[0:C, 0:P],
                                            scalar1=ms[0:C, 0:1],
                                            scalar2=rot0,
                                            op0=OP.add, op1=OP.add)
                    g2 = pps2.tile([128, NBLK], F32, tag="g2")
                    for b in range(NBLK):
                        nc.tensor.matmul(g2[:, b:b + 1],
                                         sqi[:, b * 128:(b + 1) * 128],
                                         ones64, start=True, stop=True)

                    # term 2 per quarter (drop +|mu|^2), then Ln; quarter 3
                    # recomputes its squares into the freed tag-0 tile.
                    for q in range(4):
                        if q == 3:
                            pq_t3 = pps1.tile([1, 1024], F32, tag="pq0")
                            psrq[3] = pq_t3
                            nc.vector.tensor_tensor(
                                out=sq, in0=tf[0:C, 3 * 1024:4 * 1024],
                                in1=tf[0:C, 3 * 1024:4 * 1024], op=OP.mult)
                            for h in range(2):
                                nc.tensor.matmul(
                                    psrq[3][:, h * 512:(h + 1) * 512],
                                    ones64, sq[:, h * 512:(h + 1) * 512],
                                    start=True, stop=False)
                        for h in range(2):
                            cs_ = slice(q * 1024 + h * 512,
                                        q * 1024 + h * 512 + 512)
                            nc.tensor.matmul(
                                psrq[q][:, h * 512:(h + 1) * 512],
                                n2mu, tf[0:C, cs_],
                                start=False, stop=True)
                        qs = slice(q * 1024, (q + 1) * 1024)
                        nc.scalar.activation(out=lnh[:, qs], in_=psrq[q],
                                             func=AF.Ln)
                    nc.scalar.activation(out=lng, in_=g2, func=AF.Ln)

                # zero bias that depends on the last Ln outputs: pins every
                # Exp after every Ln so walrus loads each table exactly once
                zb1 = sp.tile([1, 1], F32, tag="zb1")
                nc.vector.tensor_tensor(out=zb1, in0=lnh[0:1, P - 1:P],
                                        in1=lng[0:1, 0:1], op=OP.mult)
                zero_b = sp.tile([1, 1], F32, tag="zero_b")
                nc.vector.tensor_scalar_mul(zero_b, zb1, 0.0)

                # single table switch: all Exps (g first — it gates block 0)
                nc.scalar.activation(out=g, in_=lng, func=AF.Exp, scale=-0.5)
                ht = pro.tile([1, P], F16, tag="ht")
                hbc = pro.tile([C, P], F16, tag="hbc")
                for q in range(4):
                    qs = slice(q * 1024, (q + 1) * 1024)
                    nc.scalar.activation(out=ht[:, qs], in_=lnh[:, qs],
                                         func=AF.Exp, scale=-0.5,
                                         bias=zero_b)
                    # broadcast to C partitions: DRAM bounce + 0-stride read
                    nc.sync.dma_start(out=hbounce[0:1, qs], in_=ht[:, qs])
                    nc.sync.dma_start(
                        out=hbc[:, qs],
                        in_=hbounce[0:1, qs].broadcast_to([C, 1024]))
                    nc.vector.tensor_tensor(out=tn[:, qs], in0=tcent[:, qs],
                                            in1=hbc[:, qs], op=OP.mult)

            # ---------------- main loop ----------------
            with (
                tc.tile_pool(name="dbuf", bufs=3) as dp,
                tc.tile_pool(name="ebuf", bufs=3) as ep,
                tc.tile_pool(name="mps", bufs=1, space="PSUM") as mps,
            ):
                for b in range(NBLK):
                    lhs = ic[:, b * 128:(b + 1) * 128]
                    dotb = dp.tile([128, P], F16, tag="dotb")
                    rm4 = sp.tile([128, 4], F32, tag="rm4")
                    for q in range(4):
                        ps = mps.tile([128, QW], F32, tag=f"ps{q}")
                        for cch in range(QW // 512):
                            off = q * QW + cch * 512
                            nc.tensor.matmul(
                                ps[:, cch * 512:(cch + 1) * 512], lhs,
                                tn[:, off:off + 512], start=True, stop=True)
                        # fused: dotb = psum * g_b (fp16) + rowmax accum
                        nc.vector.tensor_scalar(
                            out=dotb[:, q * QW:(q + 1) * QW], in0=ps,
                            scalar1=g[:, b:b + 1], scalar2=None,
                            op0=OP.mult, op1=OP.max,
                            accum_out=rm4[:, q:q + 1])
                    mq = sp.tile([128, 1], F32, tag="mq")
                    nc.vector.reduce_max(out=mq, in_=rm4, axis=AX.X)
                    dd = sp.tile([128, 1], F32, tag="dd")
                    nc.vector.tensor_scalar(out=dd, in0=mq, scalar1=-1.0,
                                            scalar2=1.0 + 2.0 * EPS,
                                            op0=OP.mult, op1=OP.add)
                    a2 = sp.tile([128, 1], F32, tag="a2")
                    nc.vector.reciprocal(a2, dd)
                    bias = sp.tile([128, 1], F32, tag="bias")
                    nc.vector.tensor_scalar(out=bias, in0=a2, scalar1=-1.0,
                                            scalar2=1.0, op0=OP.mult,
                                            op1=OP.add)
                    eb = ep.tile([128, P], F16, tag="eb")
                    for h in range(2):
                        hs = slice(h * HW_, (h + 1) * HW_)
                        nc.scalar.activation(
                            out=eb[:, hs], in_=dotb[:, hs], func=AF.Exp,
                            bias=bias, scale=a2,
                            accum_out=sscol[:, 2 * b + h:2 * b + h + 1])
                        nc.sync.dma_start(
                            out=eb_out[b * 128:(b + 1) * 128, hs],
                            in_=eb[:, hs])

            nc.sync.dma_start(out=ss_out[:, :], in_=sscol)

    nc.compile()
    return nc


_NC_CACHE = {}


def _get_nc():
    if "nc" not in _NC_CACHE:
        _NC_CACHE["nc"] = build_nc()
    return _NC_CACHE["nc"]


def make_in_maps(I_features, T_features):
    I4 = np.asarray(I_features, dtype=np.float32).reshape(N, C, P)
    T4 = np.asarray(T_features, dtype=np.float32).reshape(N, C, P)
    I4 = I4.astype(np.float16)
    T4 = T4.astype(np.float16)
    in_maps = []
    for core in range(NCORES):
        n, half = core // 2, core % 2
        # rotate batches so this core's batch is flat rows 0-63; mu is
        # order-invariant. partition p holds flat rows p and p+128.
        perm = [(n + j) % N for j in range(N)]
        tf = np.ascontiguousarray(
            T4[perm].reshape(2, 128, P).transpose(1, 0, 2).reshape(128, 2 * P))
        in_maps.append({
            "t_full": tf,
            "i_own": np.ascontiguousarray(I4[n][:, half * QH:(half + 1) * QH]),
        })
    return in_maps


def core_k(eb, ss):
    """One core's partial column max [128, P] from eb [QH,P], ss [128,2*NBLK]."""
    eb3 = np.asarray(eb, dtype=np.float32).reshape(NBLK, 128, P)
    ss2 = np.asarray(ss, dtype=np.float32).reshape(128, NBLK, 2).sum(axis=2)
    rr = 1.0 / ss2                                     # [128, NBLK]
    return (eb3 * rr.T[:, :, None]).max(axis=0)        # [128, P]


def finish_host(kparts):
    """kparts: [8, 128, P] per-core partial column maxima -> scalar score."""
    ks = np.stack([np.asarray(kp, dtype=np.float64) for kp in kparts])
    kp = ks.reshape(N, 2 * 128, P).max(axis=1)      # [N, P]
    cs = kp.mean(axis=1)                            # [N]
    return np.float32(np.mean(-np.log(cs)))


def kernel(I_features, T_features, _trace=False):
    nc = _get_nc()
    in_maps = make_in_maps(I_features, T_features)
    res = run_bass_kernel_spmd(nc, in_maps, core_ids=list(range(NCORES)),
                               trace=_trace)
    score = finish_host([core_k(r["eb_out"], r["ss_out"])
                         for r in res.results])
    if _trace:
        return np.array(score, dtype=np.float32), res
    return np.array(score, dtype=np.float32)
